# revision 5
# baseline (speedup 1.0000x reference)
"""DeBERTa-style disentangled self-attention on 8 trn2 NeuronCores.

Sharding: core c handles batch b = c//4 and head-quad q = c%4 (heads 4q..4q+3),
i.e. data parallel over the batch and tensor parallel over heads for the
QKV/positional projections and attention. The output dense is column-sharded
(each core computes dense columns 256q..256q+256 for its batch, which needs the
full attention output — exchanged with one small per-head-pair AllGather), and
the LayerNorm row statistics are completed with a 16KB AllReduce.

Algebra: scores = q.kT + rel_q.kT + q.rel_kT = [q+rel_q; q] . [k; rel_k] so the
three score terms become one K=128 contraction. Softmax skips the max-subtract
(|scores*SCALE| < ~2 for these operand scales) and folds the denominator as an
extra all-ones column of V.
"""
import sys, types

sys.path.insert(0, '/opt/trn_rl_repo')


def _install_axon_hooks():
    if "antenv.axon_hooks" in sys.modules:
        return
    m = types.ModuleType("antenv.axon_hooks")
    state = {"hook": None}

    def set_axon_ntff_profile_hook(hook):
        state["hook"] = hook

    def get_axon_ntff_profile_hook():
        if state["hook"] is None:
            sys.path.insert(0, "/root/.axon_site/trn_agent_boot")
            import trn_boot
            state["hook"] = trn_boot._ntff_profile_via_ctypes("/opt/axon/libaxon_pjrt.so")
        return state["hook"]

    m.set_axon_ntff_profile_hook = set_axon_ntff_profile_hook
    m.get_axon_ntff_profile_hook = get_axon_ntff_profile_hook
    sys.modules["antenv.axon_hooks"] = m


_install_axon_hooks()

import numpy as np
import ml_dtypes

import concourse.bass as bass
import concourse.bacc as bacc
import concourse.tile as tile
import concourse.mybir as mybir
from concourse.bass_utils import run_bass_kernel_spmd
from concourse.masks import make_identity

F32 = mybir.dt.float32
F32R = mybir.dt.float32r
BF16 = mybir.dt.bfloat16
AF = mybir.ActivationFunctionType
ALU = mybir.AluOpType
AX = mybir.AxisListType

B, N, H, D = 2, 2048, 16, 64
HID = H * D
NC = 8
HPC = 4            # heads per core
DHC = HPC * D      # 256 hid-slice per core
EPS = 1e-7
SCALE = 1.0 / (3 * D) ** 0.5
GROUPS4 = [[0, 1, 2, 3], [4, 5, 6, 7]]

NCH = 4            # n-chunks for projections (512 each)
NCHW = N // NCH    # 512
JT = N // 128      # 16 j-tiles
IC = 2             # i-chunks for attention (1024 each)
ICW = N // IC      # 1024
KT = HID // 128    # 8 contraction tiles


def _build(flags):
    use_qk_bias, use_bo, use_g, use_b = flags
    nc = bacc.Bacc("TRN2", target_bir_lowering=False, debug=False, num_devices=NC)

    def din(name, shape, dt):
        return nc.dram_tensor(name, shape, dt, kind="ExternalInput").ap()

    xT = din("xT", [HID, N], F32R)
    posT = din("posT", [HID, N], F32R)
    wq = din("wq", [HID, DHC], F32R)
    wk = din("wk", [HID, DHC], F32R)
    wv = din("wv", [HID, DHC], F32R)
    wpq = din("wpq", [HID, DHC], F32R)
    wpk = din("wpk", [HID, DHC], F32R)
    wo = din("wo", [HID, DHC], BF16)
    xres = din("xres", [N, DHC], F32)
    bvp = din("bv", [DHC], F32)
    if use_qk_bias:
        qbias = din("qbias", [128, HPC], F32)   # rows 0:64 bq+bpq, 64:128 bq per head
        kbias = din("kbias", [128, HPC], F32)   # rows 0:64 bk, 64:128 bpk per head
    if use_bo:
        bop = din("bo", [DHC], F32)
    if use_g:
        gp = din("ln_g", [DHC], F32)
    if use_b:
        bp = din("ln_b", [DHC], F32)
    out = nc.dram_tensor("out", [N, DHC], F32, kind="ExternalOutput").ap()

    kt_view = lambda t: t.rearrange("(kt p) m -> p kt m", p=128)

    with tile.TileContext(nc) as tc:
        with (
            tc.tile_pool(name="const", bufs=1) as const,
            tc.tile_pool(name="qk", bufs=1) as qkp,
            tc.tile_pool(name="vb", bufs=1) as vbp,
            tc.tile_pool(name="avt", bufs=1) as avtp,
            tc.tile_pool(name="small", bufs=4) as small,
            tc.tile_pool(name="dram", bufs=1, space="DRAM") as dram,
        ):
            # ---- constants / weights ----
            w_t = {}
            for name, src in (("wq", wq), ("wk", wk), ("wv", wv),
                              ("wpq", wpq), ("wpk", wpk)):
                t = const.tile([128, KT, DHC], F32R, tag=name)
                nc.sync.dma_start(out=t, in_=kt_view(src))
                w_t[name] = t
            wo_t = const.tile([128, KT, DHC], BF16, tag="wo")
            nc.sync.dma_start(out=wo_t, in_=kt_view(wo))
            ident = const.tile([128, 128], BF16, tag="ident")
            make_identity(nc, ident)
            eps_t = const.tile([128, 1], F32, tag="eps")
            nc.vector.memset(eps_t, EPS)
            bv_rep = const.tile([128, HPC, D], F32, tag="bvrep")
            nc.sync.dma_start(
                out=bv_rep,
                in_=bass.AP(tensor=bvp.tensor, offset=bvp.offset,
                            ap=[[0, 128]] + [list(p) for p in
                                bvp.rearrange("(h d) -> h d", h=HPC).ap]))
            if use_qk_bias:
                qb_t = const.tile([128, HPC], F32, tag="qb")
                kb_t = const.tile([128, HPC], F32, tag="kb")
                nc.sync.dma_start(out=qb_t, in_=qbias)
                nc.sync.dma_start(out=kb_t, in_=kbias)

            def rep_row(src, tag):
                t = const.tile([128, DHC], F32, tag=tag)
                nc.sync.dma_start(
                    out=t,
                    in_=bass.AP(tensor=src.tensor, offset=src.offset,
                                ap=[[0, 128]] + [list(p) for p in src.ap]))
                return t

            bo_rep = rep_row(bop, "borep") if use_bo else None
            g_rep = rep_row(gp, "grep") if use_g else None
            b_rep = rep_row(bp, "brep") if use_b else None

            qcat = [qkp.tile([128, N], BF16, tag=f"qcat{h}", name=f"qcat{h}")
                    for h in range(HPC)]
            kcat = [qkp.tile([128, N], BF16, tag=f"kcat{h}", name=f"kcat{h}")
                    for h in range(HPC)]
            v_sb = vbp.tile([128, JT, HPC, D + 1], BF16, tag="v")
            nc.vector.memset(v_sb, 1.0)
            avT = [avtp.tile([128, N], BF16, tag=f"avT{pp}", name=f"avT{pp}")
                   for pp in range(2)]

            # ---- phase 1: projections ----
            with (
                tc.tile_pool(name="xtp", bufs=2) as xtp,
                tc.tile_pool(name="ppsum", bufs=1, space="PSUM") as pps,
            ):
                for nch in range(NCH):
                    ns = nch * NCHW
                    xt_c = xtp.tile([128, KT, NCHW], F32R, tag="xt")
                    nc.sync.dma_start(out=xt_c, in_=kt_view(xT)[:, :, ns:ns + NCHW])
                    pos_c = xtp.tile([128, KT, NCHW], F32R, tag="pos")
                    nc.sync.dma_start(out=pos_c, in_=kt_view(posT)[:, :, ns:ns + NCHW])
                    for pr in range(2):
                        ms = pr * 128
                        cs = slice(ns, ns + NCHW)
                        pq = pps.tile([128, NCHW], F32, tag="pq", bufs=2)
                        pk = pps.tile([128, NCHW], F32, tag="pk", bufs=2)
                        prk = pps.tile([128, NCHW], F32, tag="prk", bufs=2)
                        for kt in range(KT):
                            nc.tensor.matmul(pq, w_t["wq"][:, kt, ms:ms + 128],
                                             xt_c[:, kt, :],
                                             start=(kt == 0), stop=False)
                            nc.tensor.matmul(pk, w_t["wk"][:, kt, ms:ms + 128],
                                             xt_c[:, kt, :],
                                             start=(kt == 0), stop=(kt == KT - 1))
                            nc.tensor.matmul(prk, w_t["wpk"][:, kt, ms:ms + 128],
                                             pos_c[:, kt, :],
                                             start=(kt == 0), stop=(kt == KT - 1))
                        # evict plain q rows, then accumulate rel_q on top of pq
                        for hi in range(2):
                            h = pr * 2 + hi
                            sl = slice(64 * hi, 64 * hi + 64)
                            nc.vector.tensor_copy(out=qcat[h][64:128, cs],
                                                  in_=pq[sl, :])
                        for kt in range(KT):
                            nc.tensor.matmul(pq, w_t["wpq"][:, kt, ms:ms + 128],
                                             pos_c[:, kt, :],
                                             start=False, stop=(kt == KT - 1),
                                             skip_group_check=True)
                        for hi in range(2):
                            h = pr * 2 + hi
                            sl = slice(64 * hi, 64 * hi + 64)
                            nc.vector.tensor_copy(out=qcat[h][0:64, cs],
                                                  in_=pq[sl, :])
                            nc.vector.tensor_copy(out=kcat[h][0:64, cs],
                                                  in_=pk[sl, :])
                            nc.vector.tensor_copy(out=kcat[h][64:128, cs],
                                                  in_=prk[sl, :])
                            if use_qk_bias:
                                for tt, bt in ((qcat, qb_t), (kcat, kb_t)):
                                    nc.vector.tensor_scalar_add(
                                        out=tt[h][:, cs], in0=tt[h][:, cs],
                                        scalar1=bt[:, h:h + 1])
                    for jb in range(NCH):
                        jg = nch * NCH + jb
                        pv = pps.tile([128, DHC], F32, tag="pv", bufs=2)
                        for kt in range(KT):
                            nc.tensor.matmul(pv, xt_c[:, kt, jb * 128:jb * 128 + 128],
                                             w_t["wv"][:, kt, :],
                                             start=(kt == 0), stop=(kt == KT - 1))
                        nc.vector.tensor_add(
                            out=v_sb[:, jg, :, 0:D],
                            in0=pv.rearrange("p (h d) -> p h d", h=HPC),
                            in1=bv_rep)

            # ---- phase 2: attention ----
            with (
                tc.tile_pool(name="pb", bufs=2) as pbp,
                tc.tile_pool(name="apsum", bufs=1, space="PSUM") as aps,
            ):
                ag_in = [dram.tile([128, N], BF16, tag=f"agin{pp}", name=f"agin{pp}")
                         for pp in range(2)]
                ag_out = [dram.tile([4, 128, N], BF16, tag=f"agout{pp}",
                                    name=f"agout{pp}") for pp in range(2)]
                for h in range(HPC):
                    for ic in range(IC):
                        isb = ic * ICW
                        p_sb = pbp.tile([128, JT, ICW], BF16, tag="psb")
                        for jt in range(JT):
                            sp = aps.tile([128, ICW], F32, tag="sp", bufs=2)
                            for half in range(2):
                                hb = half * 512
                                nc.tensor.matmul(
                                    sp[:, hb:hb + 512],
                                    kcat[h][:, jt * 128:jt * 128 + 128],
                                    qcat[h][:, isb + hb:isb + hb + 512],
                                    start=True, stop=True)
                            nc.scalar.activation(out=p_sb[:, jt, :], in_=sp,
                                                 func=AF.Exp, scale=SCALE)
                        for ib in range(ICW // 128):
                            ap2 = aps.tile([128, D + 1], F32, tag="av", bufs=2)
                            for jt in range(JT):
                                nc.tensor.matmul(
                                    ap2, p_sb[:, jt, ib * 128:ib * 128 + 128],
                                    v_sb[:, jt, h, :],
                                    start=(jt == 0), stop=(jt == JT - 1))
                            r_t = small.tile([128, 1], F32, tag="r")
                            nc.vector.reciprocal(out=r_t, in_=ap2[:, D:D + 1])
                            av_t = small.tile([128, D], BF16, tag="avsb")
                            nc.vector.tensor_scalar_mul(out=av_t, in0=ap2[:, 0:D],
                                                        scalar1=r_t)
                            tp = aps.tile([64, 128], BF16, tag="tp", bufs=2)
                            nc.tensor.transpose(tp, av_t, ident)
                            gi = isb + ib * 128
                            nc.vector.tensor_copy(
                                out=avT[h // 2][64 * (h % 2):64 * (h % 2) + 64,
                                                gi:gi + 128],
                                in_=tp)
                    if h % 2 == 1:
                        pp = h // 2
                        nc.sync.dma_start(out=ag_in[pp], in_=avT[pp])
                        nc.gpsimd.collective_compute(
                            "AllGather", ALU.bypass, replica_groups=GROUPS4,
                            ins=[ag_in[pp].opt()], outs=[ag_out[pp].opt()])

            # ---- phase 3: dense + layernorm ----
            with (
                tc.tile_pool(name="dn", bufs=1) as dnp,
                tc.tile_pool(name="dsc", bufs=2) as dscp,
                tc.tile_pool(name="dpsum", bufs=1, space="PSUM") as dps,
            ):
                avfull = dnp.tile([128, KT, N], BF16, tag="avfull")
                for pp in range(2):
                    nc.sync.dma_start(
                        out=avfull.rearrange("p (s t) n -> p s t n", t=2)[:, :, pp, :],
                        in_=ag_out[pp])
                xres_sb = dnp.tile([128, JT, DHC], F32, tag="xres")
                nc.sync.dma_start(out=xres_sb,
                                  in_=xres.rearrange("(ib p) c -> p ib c", p=128))
                dense_t = dnp.tile([128, JT, DHC], F32, tag="dt")
                stats = dnp.tile([128, JT, 2], F32, tag="stats")
                for ib in range(JT):
                    pd = dps.tile([128, DHC], F32, tag="pd", bufs=2)
                    for kt in range(KT):
                        nc.tensor.matmul(pd, avfull[:, kt, ib * 128:ib * 128 + 128],
                                         wo_t[:, kt, :],
                                         start=(kt == 0), stop=(kt == KT - 1))
                    dt_i = dense_t[:, ib, :]
                    nc.vector.tensor_add(out=dt_i, in0=pd, in1=xres_sb[:, ib, :])
                    if use_bo:
                        nc.vector.tensor_add(out=dt_i, in0=dt_i, in1=bo_rep)
                    nc.vector.reduce_sum(stats[:, ib, 0:1], dt_i, axis=AX.X)
                    sq = dscp.tile([128, DHC], F32, tag="sq")
                    nc.scalar.activation(out=sq, in_=dt_i, func=AF.Square,
                                         accum_out=stats[:, ib, 1:2])
                ar_in = dram.tile([N, 2], F32, tag="arin")
                ar_out = dram.tile([N, 2], F32, tag="arout")
                nc.sync.dma_start(out=ar_in.rearrange("(ib p) s -> p ib s", p=128),
                                  in_=stats)
                nc.gpsimd.collective_compute(
                    "AllReduce", ALU.add, replica_groups=GROUPS4,
                    ins=[ar_in.opt()], outs=[ar_out.opt()])
                stats2 = dnp.tile([128, JT, 2], F32, tag="stats2")
                nc.sync.dma_start(out=stats2,
                                  in_=ar_out.rearrange("(ib p) s -> p ib s", p=128))
                inv_hid = 1.0 / HID
                for ib in range(JT):
                    m_t = small.tile([128, 1], F32, tag="m")
                    v_t = small.tile([128, 1], F32, tag="vv")
                    sq_t = small.tile([128, 1], F32, tag="sqm")
                    nc.vector.tensor_scalar_mul(out=m_t, in0=stats2[:, ib, 0:1],
                                                scalar1=inv_hid)
                    nc.vector.tensor_mul(out=sq_t, in0=m_t, in1=m_t)
                    nc.vector.tensor_scalar_mul(out=v_t, in0=stats2[:, ib, 1:2],
                                                scalar1=inv_hid)
                    nc.vector.tensor_sub(out=v_t, in0=v_t, in1=sq_t)
                    nc.scalar.activation(out=v_t, in_=v_t, func=AF.Sqrt,
                                         bias=eps_t)
                    nc.vector.reciprocal(out=v_t, in_=v_t)
                    o_t = dscp.tile([128, DHC], F32, tag="ot")
                    nc.vector.tensor_scalar(out=o_t, in0=dense_t[:, ib, :],
                                            scalar1=m_t, scalar2=v_t,
                                            op0=ALU.subtract, op1=ALU.mult)
                    if use_g:
                        nc.vector.tensor_mul(out=o_t, in0=o_t, in1=g_rep)
                    if use_b:
                        nc.vector.tensor_add(out=o_t, in0=o_t, in1=b_rep)
                    nc.sync.dma_start(
                        out=out.rearrange("(ib p) c -> p ib c", p=128)[:, ib, :],
                        in_=o_t)

    nc.compile()
    return nc


_STATE = {}


def kernel(hidden_states, Wq, bq, Wk, bk, Wv, bv, pos_emb, Wpq, bpq, Wpk, bpk,
           Wo, bo, ln_g, ln_b):
    x = np.asarray(hidden_states, np.float32)
    Wq, Wk, Wv = (np.asarray(w, np.float32) for w in (Wq, Wk, Wv))
    Wpq, Wpk, Wo = (np.asarray(w, np.float32) for w in (Wpq, Wpk, Wo))
    pos = np.asarray(pos_emb, np.float32)
    bq, bk, bv, bpq, bpk, bo = (np.asarray(v, np.float32)
                                for v in (bq, bk, bv, bpq, bpk, bo))
    ln_g, ln_b = np.asarray(ln_g, np.float32), np.asarray(ln_b, np.float32)

    use_qk_bias = bool(np.any(bq) or np.any(bk) or np.any(bpq) or np.any(bpk))
    use_bo = bool(np.any(bo))
    use_g = bool(np.any(ln_g != 1.0))
    use_b = bool(np.any(ln_b))
    flags = (use_qk_bias, use_bo, use_g, use_b)
    if flags not in _STATE:
        _STATE[flags] = _build(flags)
    nc = _STATE[flags]

    xT = [np.ascontiguousarray(x[b].T) for b in range(B)]
    posT = np.ascontiguousarray(pos.T)
    in_maps = []
    for c in range(NC):
        b, q = c // 4, c % 4
        hs = slice(DHC * q, DHC * q + DHC)
        im = {
            "xT": xT[b],
            "posT": posT,
            "wq": np.ascontiguousarray(Wq[:, hs]),
            "wk": np.ascontiguousarray(Wk[:, hs]),
            "wv": np.ascontiguousarray(Wv[:, hs]),
            "wpq": np.ascontiguousarray(Wpq[:, hs]),
            "wpk": np.ascontiguousarray(Wpk[:, hs]),
            "wo": np.ascontiguousarray(Wo[:, hs]).astype(ml_dtypes.bfloat16),
            "xres": np.ascontiguousarray(x[b][:, hs]),
            "bv": np.ascontiguousarray(bv[hs]),
        }
        if use_qk_bias:
            qb = np.zeros((128, HPC), np.float32)
            kb = np.zeros((128, HPC), np.float32)
            for hh in range(HPC):
                ds = slice(DHC * q + D * hh, DHC * q + D * hh + D)
                qb[0:64, hh] = bq[ds] + bpq[ds]
                qb[64:128, hh] = bq[ds]
                kb[0:64, hh] = bk[ds]
                kb[64:128, hh] = bpk[ds]
            im["qbias"], im["kbias"] = qb, kb
        if use_bo:
            im["bo"] = np.ascontiguousarray(bo[hs])
        if use_g:
            im["ln_g"] = np.ascontiguousarray(ln_g[hs])
        if use_b:
            im["ln_b"] = np.ascontiguousarray(ln_b[hs])
        in_maps.append(im)

    res = run_bass_kernel_spmd(nc, in_maps, list(range(NC)))
    out = np.empty((B, N, HID), np.float32)
    for c in range(NC):
        b, q = c // 4, c % 4
        out[b, :, DHC * q:DHC * q + DHC] = res.results[c]["out"]
    return out
